# Initial kernel scaffold
#
"""Expert-parallel MoE MLP + residual + LayerNorm on 8 Trainium2 NeuronCores.

Reference computes a dense all-expert MLP then masks: out[t] only depends on
expert e = mask[t].  We route: core d gets expert d's weights plus the tokens
assigned to expert d (gathered on host, zero-padded to a fixed capacity C),
computes gelu(x@w1+b1)@w2+b2, adds the residual, applies LayerNorm, and the
host scatters rows back.  No collectives needed: each token's output lives on
exactly one core.

Per-core layout (feature-major for matmul1, token-major after matmul2):
  matmul1: interT[i, t] = sum_h w1[h, i] * x[t, h]   (lhsT=w1 chunk, rhs=x^T)
  gelu+b1 fused in one ACT op (bias is per-partition in feature-major layout)
  matmul2: y[t, h] = sum_i interT[i, t] * w2[i, h]   (lhsT=interT chunk, rhs=w2)
  LayerNorm in token-major layout (reduction along the free dim).
b2 is folded into the residual operand on the host.
"""

import numpy as np
import ml_dtypes

import concourse.bacc as bacc
import concourse.mybir as mybir
import concourse.tile as tile
from concourse.bass_utils import run_bass_kernel_spmd

E, T, H, I = 8, 8192, 768, 3072
P = 128
HK, IK = H // P, I // P  # 6, 24
EPS = 1e-12
N_CORES = 8

F32 = mybir.dt.float32
BF16 = mybir.dt.bfloat16
AF = mybir.ActivationFunctionType
ALU = mybir.AluOpType


def _build(C: int, act=AF.Gelu, reps: int = 1, n_tok: int | None = None):
    """C: DRAM capacity (multiple of 128). n_tok: tokens actually computed
    (n_tok <= C); the tail beyond n_tok is padding nobody reads back."""
    if n_tok is None:
        n_tok = C
    TCN = C // P  # token chunks per core (DRAM layout)
    blocks = []
    off = 0
    while off < n_tok:
        tb = min(512, n_tok - off)
        blocks.append((off, tb))
        off += tb

    nc = bacc.Bacc(None, target_bir_lowering=False)

    xgt_d = nc.dram_tensor("xgt", [HK, P, C], BF16, kind="ExternalInput")
    xres_d = nc.dram_tensor("xres", [TCN, P, H], F32, kind="ExternalInput")
    w1_d = nc.dram_tensor("w1", [HK, P, I], BF16, kind="ExternalInput")
    b1t_d = nc.dram_tensor("b1t", [P, IK], F32, kind="ExternalInput")
    w2_d = nc.dram_tensor("w2", [IK, P, H], BF16, kind="ExternalInput")
    gb_d = nc.dram_tensor("gb", [P, 2, H], F32, kind="ExternalInput")
    out_d = nc.dram_tensor("out", [TCN, P, H], F32, kind="ExternalOutput")

    with tile.TileContext(nc) as tc:
        with (
            tc.tile_pool(name="res", bufs=1) as rpool,
            tc.tile_pool(name="acts", bufs=2) as apool,
            tc.tile_pool(name="ln", bufs=2) as lnpool,
            tc.tile_pool(name="small", bufs=4) as spool,
            tc.tile_pool(name="psA", bufs=4, space="PSUM") as ppa,
            tc.tile_pool(name="psB", bufs=2, space="PSUM") as ppb,
        ):
            epssb = rpool.tile([P, 1], F32)
            nc.gpsimd.memset(epssb[:], EPS)
            b1sb = rpool.tile([P, IK], F32)
            gbsb = rpool.tile([P, 2, H], F32)
            nc.sync.dma_start(b1sb[:], b1t_d[:])
            nc.sync.dma_start(gbsb[:], gb_d[:])

            for _rep in range(reps):
                # Per-chunk tiles so DMA->compute deps are exact: the first
                # matmul fires as soon as w1[0]/xgt[0] land, not after 13MB.
                w1sb = [rpool.tile([P, I], BF16, tag=f"w1_{k}", name=f"w1sb{k}") for k in range(HK)]
                xgtsb = [rpool.tile([P, C], BF16, tag=f"xgt_{k}", name=f"xgtsb{k}") for k in range(HK)]
                w2sb = [rpool.tile([P, H], BF16, tag=f"w2_{k}", name=f"w2sb{k}") for k in range(IK)]
                xressb = [rpool.tile([P, H], F32, tag=f"xres_{c}", name=f"xressb{c}") for c in range(TCN)]

                for k in range(HK):
                    # halves on separate queues: first matmul waits ~half as long
                    nc.sync.dma_start(w1sb[k][:, : I // 2], w1_d[k][:, : I // 2])
                    nc.sync.dma_start(w1sb[k][:, I // 2 :], w1_d[k][:, I // 2 :])
                    nc.sync.dma_start(xgtsb[k][:], xgt_d[k])

                for bi, (boff, tb) in enumerate(blocks):
                    interT = apool.tile([P, IK, 512], BF16, tag="interT")
                    for m in range(IK):
                        if bi == 0 and m == 10:
                            # w2/xres issued mid-block-0 so they don't steal
                            # HBM bandwidth from the critical w1/xgt path, yet
                            # land before stage B needs them.
                            for k2 in range(IK):
                                nc.sync.dma_start(w2sb[k2][:], w2_d[k2])
                            for c in range(TCN):
                                nc.sync.dma_start(xressb[c][:], xres_d[c])
                        ps = ppa.tile([P, 512], F32, tag="psA")
                        for k in range(HK):
                            nc.tensor.matmul(
                                ps[:, :tb],
                                w1sb[k][:, m * P : (m + 1) * P],
                                xgtsb[k][:, boff : boff + tb],
                                start=(k == 0),
                                stop=(k == HK - 1),
                            )
                        nc.scalar.activation(
                            interT[:, m, :tb], ps[:, :tb], act, bias=b1sb[:, m : m + 1]
                        )

                    for tci in range((tb + P - 1) // P):
                        tcg = boff // P + tci
                        toff = tci * P
                        tw = min(P, tb - toff)
                        psy = ppb.tile([P, H], F32, tag="psB")
                        for n0, nw in ((0, 512), (512, 256)):
                            for k in range(IK):
                                nc.tensor.matmul(
                                    psy[:tw, n0 : n0 + nw],
                                    interT[:, k, toff : toff + tw],
                                    w2sb[k][:, n0 : n0 + nw],
                                    start=(k == 0),
                                    stop=(k == IK - 1),
                                )
                        # LayerNorm over H (free dim). (tensor_tensor_reduce
                        # would fuse the residual add with the row sum, but it
                        # crashes the exec unit on hw — use add + reduce_sum.)
                        x = lnpool.tile([P, H], F32, tag="x")
                        nc.vector.tensor_add(x[:tw], psy[:tw], xressb[tcg][:tw])
                        s1 = spool.tile([P, 1], F32, tag="s1")
                        nc.vector.reduce_sum(s1[:tw], x[:tw], axis=mybir.AxisListType.X)
                        sq = lnpool.tile([P, H], F32, tag="sq")
                        s2 = spool.tile([P, 1], F32, tag="s2")
                        nc.scalar.activation(sq[:tw], x[:tw], AF.Square, accum_out=s2[:tw])
                        mu = spool.tile([P, 1], F32, tag="mu")
                        nc.vector.tensor_scalar_mul(mu[:tw], s1[:tw], 1.0 / H)
                        ex2 = spool.tile([P, 1], F32, tag="ex2")
                        nc.vector.tensor_scalar_mul(ex2[:tw], s2[:tw], 1.0 / H)
                        mu2 = spool.tile([P, 1], F32, tag="mu2")
                        nc.vector.tensor_mul(mu2[:tw], mu[:tw], mu[:tw])
                        var = spool.tile([P, 1], F32, tag="var")
                        nc.vector.tensor_sub(var[:tw], ex2[:tw], mu2[:tw])
                        std = spool.tile([P, 1], F32, tag="std")
                        nc.scalar.activation(std[:tw], var[:tw], AF.Sqrt, bias=epssb[:tw])
                        rs = spool.tile([P, 1], F32, tag="rs")
                        nc.vector.reciprocal(rs[:tw], std[:tw])
                        nmr = spool.tile([P, 1], F32, tag="nmr")
                        nc.vector.tensor_scalar(
                            nmr[:tw], mu[:tw], rs[:tw], -1.0, op0=ALU.mult, op1=ALU.mult
                        )
                        o = lnpool.tile([P, H], F32, tag="o")
                        nc.vector.tensor_scalar(
                            o[:tw], x[:tw], rs[:tw], nmr[:tw], op0=ALU.mult, op1=ALU.add
                        )
                        nc.vector.tensor_mul(o[:tw], o[:tw], gbsb[:tw, 0, :])
                        nc.vector.tensor_add(o[:tw], o[:tw], gbsb[:tw, 1, :])
                        nc.sync.dma_start(out_d[tcg][:tw], o[:tw])

    nc.finalize()
    return nc


_NC_CACHE: dict[tuple, object] = {}


def _get_nc(C: int, n_tok: int, reps: int = 1):
    key = (C, n_tok, reps)
    if key not in _NC_CACHE:
        _NC_CACHE[key] = _build(C, reps=reps, n_tok=n_tok)
    return _NC_CACHE[key]


def _prepare(hidden_states, mask, w1, b1, w2, b2, ln_gamma, ln_beta, reps=1):
    hs = np.asarray(hidden_states, dtype=np.float32)
    mk = np.asarray(mask).reshape(-1).astype(np.int64)
    w1 = np.asarray(w1, dtype=np.float32)
    b1 = np.asarray(b1, dtype=np.float32)
    w2 = np.asarray(w2, dtype=np.float32)
    b2 = np.asarray(b2, dtype=np.float32)
    g = np.asarray(ln_gamma, dtype=np.float32)
    bt = np.asarray(ln_beta, dtype=np.float32)

    idxs = [np.nonzero(mk == e)[0] for e in range(E)]
    max_n = max(len(ix) for ix in idxs)
    C = max(256, -(-max_n // P) * P)  # DRAM capacity: multiple of 128
    n_tok = max(256, max_n)  # tokens actually computed
    nc = _get_nc(C, n_tok, reps)
    TCN = C // P

    gb = np.empty((P, 2, H), dtype=np.float32)
    gb[:, 0, :] = g[None, :]
    gb[:, 1, :] = bt[None, :]

    hs2 = hs.reshape(T, H)
    in_maps = []
    for e in range(E):
        ix = idxs[e]
        xg = np.zeros((C, H), dtype=np.float32)
        xg[: len(ix)] = hs2[ix]
        xgt = np.ascontiguousarray(xg.T).astype(ml_dtypes.bfloat16).reshape(HK, P, C)
        xres = (xg + b2[e][None, :]).reshape(TCN, P, H)
        in_maps.append(
            {
                "xgt": xgt,
                "xres": xres,
                "w1": w1[e].astype(ml_dtypes.bfloat16).reshape(HK, P, I),
                "b1t": np.ascontiguousarray(b1[e].reshape(IK, P).T),
                "w2": w2[e].astype(ml_dtypes.bfloat16).reshape(IK, P, H),
                "gb": gb,
            }
        )

    return nc, in_maps, idxs, C


def _scatter(res, idxs, C):
    out = np.empty((T, H), dtype=np.float32)
    for e in range(E):
        ix = idxs[e]
        out[ix] = res.results[e]["out"].reshape(C, H)[: len(ix)]
    return out.reshape(1, T, H)


def kernel(**inputs):
    nc, in_maps, idxs, C = _prepare(**inputs)
    res = run_bass_kernel_spmd(nc, in_maps, list(range(N_CORES)))
    return _scatter(res, idxs, C)



# revision 2
# speedup vs baseline: 1.5767x; 1.5767x over previous
"""Expert-parallel MoE MLP + residual + LayerNorm on 8 Trainium2 NeuronCores.

Reference computes a dense all-expert MLP then masks: out[t] only depends on
expert e = mask[t].  We route: core d gets expert d's weights plus the tokens
assigned to expert d (gathered on host, zero-padded to a fixed capacity C),
computes gelu(x@w1+b1)@w2+b2, adds the residual, applies LayerNorm, and the
host scatters rows back.  No collectives needed: each token's output lives on
exactly one core.

fp8 mode (default): both matmuls run in fp8e4 with DoubleRow perf mode (two
128-deep k-tiles per instruction, 2x PE throughput).  Weights are scaled by
64 on the host so they sit in fp8's normal range; gelu undoes the scale via
the ACT unit's input scale (gelu(ps/64 + b1)), and the second matmul's x64
output scale is absorbed by also scaling the residual by 64 — LayerNorm is
scale-invariant, so the final output is unchanged.

Per-core layout (feature-major for matmul1, token-major after matmul2):
  matmul1: interT[i, t] = sum_h w1[h, i] * x[t, h]   (lhsT=w1 chunk, rhs=x^T)
  gelu+b1 fused in one ACT op (bias is per-partition in feature-major layout)
  matmul2: y[t, h] = sum_i interT[i, t] * w2[i, h]   (lhsT=interT chunk, rhs=w2)
  LayerNorm in token-major layout (reduction along the free dim).
b2 is folded into the residual operand on the host.
"""

import numpy as np
import ml_dtypes

import concourse.bacc as bacc
import concourse.mybir as mybir
import concourse.tile as tile
from concourse.bass_utils import run_bass_kernel_spmd

E, T, H, I = 8, 8192, 768, 3072
P = 128
HK, IK = H // P, I // P  # 6, 24
EPS = 1e-12
N_CORES = 8
WSCALE = 64.0  # fp8 weight pre-scale (power of 2)

F32 = mybir.dt.float32
BF16 = mybir.dt.bfloat16
FP8 = mybir.dt.float8e4
AF = mybir.ActivationFunctionType
ALU = mybir.AluOpType
DR = mybir.MatmulPerfMode.DoubleRow

MODE = "fp8"  # "fp8" | "bf16"


def _ln_block(nc, spool, lnpool, epssb, gbsb, psy, xres, tw):
    """LayerNorm over the free dim of psy+xres -> returns output tile."""
    x = lnpool.tile([P, H], F32, tag="x")
    nc.vector.tensor_add(x[:tw], psy[:tw], xres[:tw])
    s1 = spool.tile([P, 1], F32, tag="s1")
    nc.vector.reduce_sum(s1[:tw], x[:tw], axis=mybir.AxisListType.X)
    sq = lnpool.tile([P, H], F32, tag="sq")
    s2 = spool.tile([P, 1], F32, tag="s2")
    nc.scalar.activation(sq[:tw], x[:tw], AF.Square, accum_out=s2[:tw])
    mu = spool.tile([P, 1], F32, tag="mu")
    nc.vector.tensor_scalar_mul(mu[:tw], s1[:tw], 1.0 / H)
    ex2 = spool.tile([P, 1], F32, tag="ex2")
    nc.vector.tensor_scalar_mul(ex2[:tw], s2[:tw], 1.0 / H)
    mu2 = spool.tile([P, 1], F32, tag="mu2")
    nc.vector.tensor_mul(mu2[:tw], mu[:tw], mu[:tw])
    var = spool.tile([P, 1], F32, tag="var")
    nc.vector.tensor_sub(var[:tw], ex2[:tw], mu2[:tw])
    std = spool.tile([P, 1], F32, tag="std")
    nc.scalar.activation(std[:tw], var[:tw], AF.Sqrt, bias=epssb[:tw])
    rs = spool.tile([P, 1], F32, tag="rs")
    nc.vector.reciprocal(rs[:tw], std[:tw])
    nmr = spool.tile([P, 1], F32, tag="nmr")
    nc.vector.tensor_scalar(
        nmr[:tw], mu[:tw], rs[:tw], -1.0, op0=ALU.mult, op1=ALU.mult
    )
    o = lnpool.tile([P, H], F32, tag="o")
    nc.vector.tensor_scalar(
        o[:tw], x[:tw], rs[:tw], nmr[:tw], op0=ALU.mult, op1=ALU.add
    )
    nc.vector.tensor_mul(o[:tw], o[:tw], gbsb[:tw, 0, :])
    nc.vector.tensor_add(o[:tw], o[:tw], gbsb[:tw, 1, :])
    return o


def _build_fp8(C: int, reps: int = 1, n_tok: int | None = None):
    if n_tok is None:
        n_tok = C
    TCN = C // P
    blocks = []
    off = 0
    while off < n_tok:
        tb = min(512, n_tok - off)
        blocks.append((off, tb))
        off += tb

    nc = bacc.Bacc(None, target_bir_lowering=False)

    xgt_d = nc.dram_tensor("xgt", [P, HK, C], FP8, kind="ExternalInput")
    xres_d = nc.dram_tensor("xres", [TCN, P, H], F32, kind="ExternalInput")
    w1_d = nc.dram_tensor("w1", [P, HK, I], FP8, kind="ExternalInput")
    b1t_d = nc.dram_tensor("b1t", [P, IK], F32, kind="ExternalInput")
    w2_d = nc.dram_tensor("w2", [P, IK, H], FP8, kind="ExternalInput")
    gb_d = nc.dram_tensor("gb", [P, 2, H], F32, kind="ExternalInput")
    out_d = nc.dram_tensor("out", [TCN, P, H], F32, kind="ExternalOutput")

    with tile.TileContext(nc) as tc:
        with (
            tc.tile_pool(name="res", bufs=1) as rpool,
            tc.tile_pool(name="acts", bufs=2) as apool,
            tc.tile_pool(name="ln", bufs=2) as lnpool,
            tc.tile_pool(name="small", bufs=4) as spool,
            tc.tile_pool(name="psA", bufs=4, space="PSUM") as ppa,
            tc.tile_pool(name="psB", bufs=2, space="PSUM") as ppb,
        ):
            epssb = rpool.tile([P, 1], F32)
            nc.gpsimd.memset(epssb[:], EPS)
            b1sb = rpool.tile([P, IK], F32)
            gbsb = rpool.tile([P, 2, H], F32)
            nc.sync.dma_start(b1sb[:], b1t_d[:])
            nc.sync.dma_start(gbsb[:], gb_d[:])

            for _rep in range(reps):
                w1sb = rpool.tile([P, HK, I], FP8, tag="w1", name="w1sb")
                xgtsb = rpool.tile([P, HK, C], FP8, tag="xgt", name="xgtsb")
                w2sb = rpool.tile([P, IK, H], FP8, tag="w2", name="w2sb")
                xressb = [
                    rpool.tile([P, H], F32, tag=f"xres_{c}", name=f"xressb{c}")
                    for c in range(TCN)
                ]

                # startup-critical order: xgt first, then w1 in m-quarters so
                # chain m=0 can fire after ~1.4MB instead of ~3.1MB.
                nc.sync.dma_start(xgtsb[:], xgt_d[:])
                IQ = I // 4
                for q in range(4):
                    nc.sync.dma_start(
                        w1sb[:, :, q * IQ : (q + 1) * IQ],
                        w1_d[:, :, q * IQ : (q + 1) * IQ],
                    )

                for bi, (boff, tb) in enumerate(blocks):
                    interT = apool.tile([P, IK, 512], FP8, tag="interT")
                    for m in range(IK):
                        if bi == 0 and m == 10:
                            # w2/xres land well before stage B needs them but
                            # don't steal HBM bandwidth from the w1/xgt path.
                            nc.sync.dma_start(w2sb[:], w2_d[:])
                            for c in range(TCN):
                                nc.sync.dma_start(xressb[c][:], xres_d[c])
                        ps = ppa.tile([P, 512], F32, tag="psA")
                        for kk in range(HK // 2):
                            nc.tensor.matmul(
                                ps[:, :tb],
                                w1sb[:, 2 * kk : 2 * kk + 2, m * P : (m + 1) * P],
                                xgtsb[:, 2 * kk : 2 * kk + 2, boff : boff + tb],
                                start=(kk == 0),
                                stop=(kk == HK // 2 - 1),
                                perf_mode=DR,
                            )
                        nc.scalar.activation(
                            interT[:, m, :tb],
                            ps[:, :tb],
                            AF.Gelu,
                            bias=b1sb[:, m : m + 1],
                            scale=1.0 / WSCALE,
                        )

                    for tci in range((tb + P - 1) // P):
                        tcg = boff // P + tci
                        toff = tci * P
                        tw = min(P, tb - toff)
                        psy = ppb.tile([P, H], F32, tag="psB")
                        for n0, nw in ((0, 512), (512, 256)):
                            for kk in range(IK // 2):
                                nc.tensor.matmul(
                                    psy[:tw, n0 : n0 + nw],
                                    interT[:, 2 * kk : 2 * kk + 2, toff : toff + tw],
                                    w2sb[:, 2 * kk : 2 * kk + 2, n0 : n0 + nw],
                                    start=(kk == 0),
                                    stop=(kk == IK // 2 - 1),
                                    perf_mode=DR,
                                )
                        o = _ln_block(
                            nc, spool, lnpool, epssb, gbsb, psy, xressb[tcg], tw
                        )
                        nc.sync.dma_start(out_d[tcg][:tw], o[:tw])

    nc.finalize()
    return nc


def _build_bf16(C: int, act=AF.Gelu, reps: int = 1, n_tok: int | None = None):
    if n_tok is None:
        n_tok = C
    TCN = C // P
    blocks = []
    off = 0
    while off < n_tok:
        tb = min(512, n_tok - off)
        blocks.append((off, tb))
        off += tb

    nc = bacc.Bacc(None, target_bir_lowering=False)

    xgt_d = nc.dram_tensor("xgt", [HK, P, C], BF16, kind="ExternalInput")
    xres_d = nc.dram_tensor("xres", [TCN, P, H], F32, kind="ExternalInput")
    w1_d = nc.dram_tensor("w1", [HK, P, I], BF16, kind="ExternalInput")
    b1t_d = nc.dram_tensor("b1t", [P, IK], F32, kind="ExternalInput")
    w2_d = nc.dram_tensor("w2", [IK, P, H], BF16, kind="ExternalInput")
    gb_d = nc.dram_tensor("gb", [P, 2, H], F32, kind="ExternalInput")
    out_d = nc.dram_tensor("out", [TCN, P, H], F32, kind="ExternalOutput")

    with tile.TileContext(nc) as tc:
        with (
            tc.tile_pool(name="res", bufs=1) as rpool,
            tc.tile_pool(name="acts", bufs=2) as apool,
            tc.tile_pool(name="ln", bufs=2) as lnpool,
            tc.tile_pool(name="small", bufs=4) as spool,
            tc.tile_pool(name="psA", bufs=4, space="PSUM") as ppa,
            tc.tile_pool(name="psB", bufs=2, space="PSUM") as ppb,
        ):
            epssb = rpool.tile([P, 1], F32)
            nc.gpsimd.memset(epssb[:], EPS)
            b1sb = rpool.tile([P, IK], F32)
            gbsb = rpool.tile([P, 2, H], F32)
            nc.sync.dma_start(b1sb[:], b1t_d[:])
            nc.sync.dma_start(gbsb[:], gb_d[:])

            for _rep in range(reps):
                w1sb = [rpool.tile([P, I], BF16, tag=f"w1_{k}", name=f"w1sb{k}") for k in range(HK)]
                xgtsb = [rpool.tile([P, C], BF16, tag=f"xgt_{k}", name=f"xgtsb{k}") for k in range(HK)]
                w2sb = [rpool.tile([P, H], BF16, tag=f"w2_{k}", name=f"w2sb{k}") for k in range(IK)]
                xressb = [rpool.tile([P, H], F32, tag=f"xres_{c}", name=f"xressb{c}") for c in range(TCN)]

                for k in range(HK):
                    nc.sync.dma_start(w1sb[k][:, : I // 2], w1_d[k][:, : I // 2])
                    nc.sync.dma_start(w1sb[k][:, I // 2 :], w1_d[k][:, I // 2 :])
                    nc.sync.dma_start(xgtsb[k][:], xgt_d[k])

                for bi, (boff, tb) in enumerate(blocks):
                    interT = apool.tile([P, IK, 512], BF16, tag="interT")
                    for m in range(IK):
                        if bi == 0 and m == 10:
                            for k2 in range(IK):
                                nc.sync.dma_start(w2sb[k2][:], w2_d[k2])
                            for c in range(TCN):
                                nc.sync.dma_start(xressb[c][:], xres_d[c])
                        ps = ppa.tile([P, 512], F32, tag="psA")
                        for k in range(HK):
                            nc.tensor.matmul(
                                ps[:, :tb],
                                w1sb[k][:, m * P : (m + 1) * P],
                                xgtsb[k][:, boff : boff + tb],
                                start=(k == 0),
                                stop=(k == HK - 1),
                            )
                        nc.scalar.activation(
                            interT[:, m, :tb], ps[:, :tb], act, bias=b1sb[:, m : m + 1]
                        )

                    for tci in range((tb + P - 1) // P):
                        tcg = boff // P + tci
                        toff = tci * P
                        tw = min(P, tb - toff)
                        psy = ppb.tile([P, H], F32, tag="psB")
                        for n0, nw in ((0, 512), (512, 256)):
                            for k in range(IK):
                                nc.tensor.matmul(
                                    psy[:tw, n0 : n0 + nw],
                                    interT[:, k, toff : toff + tw],
                                    w2sb[k][:, n0 : n0 + nw],
                                    start=(k == 0),
                                    stop=(k == IK - 1),
                                )
                        o = _ln_block(
                            nc, spool, lnpool, epssb, gbsb, psy, xressb[tcg], tw
                        )
                        nc.sync.dma_start(out_d[tcg][:tw], o[:tw])

    nc.finalize()
    return nc


_NC_CACHE: dict[tuple, object] = {}


def _get_nc(C: int, n_tok: int, reps: int = 1):
    key = (MODE, C, n_tok, reps)
    if key not in _NC_CACHE:
        build = _build_fp8 if MODE == "fp8" else _build_bf16
        _NC_CACHE[key] = build(C, reps=reps, n_tok=n_tok)
    return _NC_CACHE[key]


def _prepare(hidden_states, mask, w1, b1, w2, b2, ln_gamma, ln_beta, reps=1):
    hs = np.asarray(hidden_states, dtype=np.float32)
    mk = np.asarray(mask).reshape(-1).astype(np.int64)
    w1 = np.asarray(w1, dtype=np.float32)
    b1 = np.asarray(b1, dtype=np.float32)
    w2 = np.asarray(w2, dtype=np.float32)
    b2 = np.asarray(b2, dtype=np.float32)
    g = np.asarray(ln_gamma, dtype=np.float32)
    bt = np.asarray(ln_beta, dtype=np.float32)

    idxs = [np.nonzero(mk == e)[0] for e in range(E)]
    max_n = max(len(ix) for ix in idxs)
    C = max(256, -(-max_n // P) * P)  # DRAM capacity: multiple of 128
    n_tok = max(256, max_n)  # tokens actually computed
    nc = _get_nc(C, n_tok, reps)
    TCN = C // P

    gb = np.empty((P, 2, H), dtype=np.float32)
    gb[:, 0, :] = g[None, :]
    gb[:, 1, :] = bt[None, :]

    hs2 = hs.reshape(T, H)
    in_maps = []
    for e in range(E):
        ix = idxs[e]
        xg = np.zeros((C, H), dtype=np.float32)
        xg[: len(ix)] = hs2[ix]
        if MODE == "fp8":
            # [P, HK, C]: xgt[p, k, t] = x[t, k*128+p]
            xgt = np.ascontiguousarray(
                xg.T.reshape(HK, P, C).transpose(1, 0, 2)
            ).astype(ml_dtypes.float8_e4m3)
            xres = ((xg + b2[e][None, :]) * WSCALE).reshape(TCN, P, H)
            # [P, HK, I]: w1[p, k, i] = w1[k*128+p, i], scaled
            w1e = np.ascontiguousarray(
                (w1[e] * WSCALE).reshape(HK, P, I).transpose(1, 0, 2)
            ).astype(ml_dtypes.float8_e4m3)
            # [P, IK, H]: w2[p, k, h] = w2[k*128+p, h], scaled
            w2e = np.ascontiguousarray(
                (w2[e] * WSCALE).reshape(IK, P, H).transpose(1, 0, 2)
            ).astype(ml_dtypes.float8_e4m3)
        else:
            xgt = (
                np.ascontiguousarray(xg.T)
                .astype(ml_dtypes.bfloat16)
                .reshape(HK, P, C)
            )
            xres = (xg + b2[e][None, :]).reshape(TCN, P, H)
            w1e = w1[e].astype(ml_dtypes.bfloat16).reshape(HK, P, I)
            w2e = w2[e].astype(ml_dtypes.bfloat16).reshape(IK, P, H)
        in_maps.append(
            {
                "xgt": xgt,
                "xres": xres,
                "w1": w1e,
                "b1t": np.ascontiguousarray(b1[e].reshape(IK, P).T),
                "w2": w2e,
                "gb": gb,
            }
        )

    return nc, in_maps, idxs, C


def _scatter(res, idxs, C):
    out = np.empty((T, H), dtype=np.float32)
    for e in range(E):
        ix = idxs[e]
        out[ix] = res.results[e]["out"].reshape(C, H)[: len(ix)]
    return out.reshape(1, T, H)


def kernel(**inputs):
    nc, in_maps, idxs, C = _prepare(**inputs)
    res = run_bass_kernel_spmd(nc, in_maps, list(range(N_CORES)))
    return _scatter(res, idxs, C)
